# revision 45
# baseline (speedup 1.0000x reference)
"""LongFormer sliding-window attention on 8 Trainium2 NeuronCores.

Sharding: batch*heads data-parallel. 24 (batch, head) pairs -> 8 cores,
each core owns one batch (core//4) and 3 consecutive heads (3*(core%4)).
No collectives.

Per-core kernel (v7):
  - All matmuls run in fp16 (1 PE cycle/row at any moving width);
    accumulation in f32 PSUM.
  - x arrives host-transposed as fp16 [768, 4096]; Q,K land in
    transposed SBUF layout qkT (3 groups of 128 partitions: [q_h0|q_h1],
    [k_h0|k_h1], [q_h2|k_h2]; k_h2 is DMA-shifted (SWDGE queue, so the
    hardware DGE pipe never blocks on it) to partitions 0:64 so
    attention matmul operands share a base partition).  V is projected
    token-major [tok, 3*65] in two half-stripe pieces; the softmax
    denominator ones column and bv are added by the PSUM->SBUF copy.
  - Scores for a 256-query chunk are computed TRANSPOSED [kpos, q] into
    a 3-bank PSUM tile laid out [t1..t4 | corner_lo | corner_hi], so
    softmax is ONE 1280-col exp per head; triangle masks are three f16
    DVE multiplies.  Edge chunks (0, 15) park their single corner
    adjacent to the valid interior so their exp stays one op and their
    two same-orientation masks fuse into one strided multiply.
  - Each round projects the K groups (g1, g2+kh2 shift) before Q01:
    chunk 2m-1's scores gate on K(m) but its queries live in stripe
    m-1, so the Q projection trails as PE filler.
  - PV for all 3 heads of a chunk accumulates into ONE [128,390] PSUM
    tile ([out|denom] x 3 heads x 2 query-halves); softmax finish is a
    single 6-wide reciprocal + a single broadcast tensor_tensor scale.
  - PSUM: 2 x 3-bank score tiles + 2 x 1-bank proj/V/PV tiles = 8 banks.
  - Emission interleaves projection pieces with per-head attention so
    the PE always has dependency-free work while Act chews softmax;
    attention lags projection by one stripe; round 7 carries chunk 14 so
    only chunk 15 trails the last projection, finishing with one fused
    scale and a single SP output DMA (one HWDGE+DGE latency on the
    final chain).  Output rows are padded 192->256 cols so every out
    DMA moves 512B elements at full bus rate (sub-512B elements pay a
    2x latency multiplier); the host slices the padding off.
  - Startup DMAs are 1/2/3-k-tile pieces ordered so the first
    projection matmul starts ~4us in and never starves thereafter;
    V(0) is projected right after stripe 0 so round 1 carries no extra
    V work; stripe prefetch is gated by a 2-deep x-tile ring so its
    long transfers never queue ahead of the latency-critical kh2
    shifts on the globally-serial DMA engines.
"""

import sys

import numpy as np

sys.path.insert(0, "/opt/trn_rl_repo")

import concourse.bass as bass  # noqa: E402
import concourse.tile as tile  # noqa: E402
from concourse import bacc, mybir  # noqa: E402
from concourse import bass_utils  # noqa: E402

B, S, E = 2, 4096, 768
H, D = 12, 64
W2 = 256            # one-sided window w
C = S // W2         # 16 chunks of 256 queries
HPC = 3             # heads per core
N_CORES = 8

f32 = mybir.dt.float32
f16 = mybir.dt.float16

KT = 6              # 768 = 6 k-tiles of 128
NT = 8              # 4096 = 8 stripes of 512 tokens
VW = 65 * HPC       # packed v width: 3 heads x (64 dims + ones col) = 195
CLO, CHI, PO = 1024, 1152, 1280   # st-tile col offsets past the interior
AEXP = mybir.ActivationFunctionType.Exp
ADD = mybir.AluOpType.add
MUL = mybir.AluOpType.mult


def _build_body(tc, aps):
    nc = tc.nc
    (xt_d, wqk_d, bqk_d, wv_d, bvt_d, masks_d, out_d) = aps

    from contextlib import ExitStack
    ctx = ExitStack()
    sb = ctx.enter_context(tc.tile_pool(name="sb", bufs=1))
    xnat_p = ctx.enter_context(tc.tile_pool(name="xnat", bufs=2))
    e_p = ctx.enter_context(tc.tile_pool(name="ep", bufs=9))
    out_p = ctx.enter_context(tc.tile_pool(name="outp", bufs=6))
    ps_s = ctx.enter_context(tc.tile_pool(name="pss", bufs=2, space="PSUM"))
    ps_p = ctx.enter_context(tc.tile_pool(name="psp", bufs=2, space="PSUM"))

    # ---- persistent SBUF tensors ----
    wqk = sb.tile([128, KT * 384], f16, tag="wqk")
    qkT = sb.tile([128, 3 * S], f16, tag="qkT")
    kh2 = sb.tile([128, S], f16, tag="kh2")     # k_h2 shifted to parts 0:64
    vsb = sb.tile([128, 2 * C * VW], f16, tag="vsb")    # 32 row-tiles
    mask = sb.tile([128, 256], f16, tag="mask")    # [tril | triu]
    wv = sb.tile([128, KT * VW], f16, tag="wv")
    bvt = sb.tile([128, 2 * VW], f16, tag="bvt")  # [bv|1]x3 heads, x2 rts
    bqk = sb.tile([128, 3], f32, tag="bqk")

    def dma_stripe16(m):
        xTn = xnat_p.tile([128, KT * 512], f16, tag="xTn", name="xTn")
        nc.sync.dma_start(
            xTn[:].rearrange("p (k c) -> p k c", k=KT),
            xt_d[:, m * 512:(m + 1) * 512].rearrange(
                "(k p) c -> p k c", p=128))
        return xTn

    def q_slice(hi, lo, n):
        if hi < 2:
            return qkT[64 * hi:64 * hi + 64, lo:lo + n]
        return qkT[0:64, 2 * S + lo:2 * S + lo + n]

    def k_slice(hi, lo, n):
        if hi < 2:
            return qkT[64 * hi:64 * hi + 64, S + lo:S + lo + n]
        return kh2[0:64, lo:lo + n]

    def proj_qk_copy(m, sg, g, off):
        nc.vector.tensor_scalar_add(
            qkT[:, g * S + m * 512: g * S + m * 512 + 512],
            sg[:, off:off + 512], bqk[:, g:g + 1])
        if g == 2:
            # k_h2 lives at parts 64:128 of G2; matmul operands must
            # share a base partition: shift to parts 0:64 via an SBUF
            # DMA on the software DGE queue (Pool), which keeps the
            # hardware DGE pipe free and never head-of-line blocks SP.
            nc.gpsimd.dma_start(
                kh2[0:64, m * 512:(m + 1) * 512],
                qkT[64:128, 2 * S + m * 512: 2 * S + (m + 1) * 512])

    def proj_qk_stripe0(x):
        # one PSUM tile PER group (PSUM deps are tile-granular: sharing
        # a tile would serialize each group's matmuls behind the
        # previous group's copy); k-tiles 0-2 go k-tile-major across
        # the groups (consume each DMA piece as it lands, PE busy from
        # first arrival), k-tiles 3-5 group-major so each group's stop
        # lands early and its PSUM->SBUF copy overlaps the rest.
        s0 = ps_s.tile([128, 1536], f32, tag="S", name="s0")
        s1 = ps_s.tile([128, 1536], f32, tag="S", name="s1")
        pc0 = ps_p.tile([128, 512], f32, tag="P", name="pc0")
        gts = (s0, s1, pc0)
        for kt in range(3):
            for g in range(3):
                nc.tensor.matmul(
                    gts[g][:, 0:512],
                    wqk[:, kt * 384 + g * 128: kt * 384 + g * 128 + 128],
                    x[:, kt * 512:(kt + 1) * 512],
                    start=(kt == 0), stop=False,
                )
        for g in range(3):
            for kt in range(3, KT):
                nc.tensor.matmul(
                    gts[g][:, 0:512],
                    wqk[:, kt * 384 + g * 128: kt * 384 + g * 128 + 128],
                    x[:, kt * 512:(kt + 1) * 512],
                    start=False, stop=(kt == KT - 1),
                )
            proj_qk_copy(0, gts[g], g, 0)

    def proj_qk_g(m, x, g, name):
        pt = ps_p.tile([128, 512], f32, tag="P", name=name)
        for kt in range(KT):
            nc.tensor.matmul(
                pt[:, 0:512],
                wqk[:, kt * 384 + g * 128: kt * 384 + g * 128 + 128],
                x[:, kt * 512:(kt + 1) * 512],
                start=(kt == 0), stop=(kt == KT - 1),
            )
        proj_qk_copy(m, pt, g, 0)

    def proj_qk_k(m, x):
        # K groups first: chunk 2m-1's scores gate on K(m) but its
        # queries live in stripe m-1, so Q01 can trail as PE filler
        proj_qk_g(m, x, 1, "pb")
        proj_qk_g(m, x, 2, "pc")

    def proj_qk_q(m, x):
        proj_qk_g(m, x, 0, "pa")

    def proj_v(m, xTn, half):
        # 2 V row-tiles of 128 tokens in one 1-bank PSUM tile
        vt = ps_p.tile([128, 512], f32, tag="P", name="vt")
        for rl in range(2):
            for kt in range(KT):
                nc.tensor.matmul(
                    vt[:, rl * 195:rl * 195 + VW],
                    xTn[:, kt * 512 + (2 * half + rl) * 128:
                        kt * 512 + (2 * half + rl) * 128 + 128],
                    wv[:, kt * VW:(kt + 1) * VW],
                    start=(kt == 0), stop=(kt == KT - 1),
                )
        rt = m * 4 + 2 * half
        # adds bv and writes the constant 1.0 denominator columns
        nc.vector.tensor_tensor(
            vsb[:, rt * VW:(rt + 2) * VW], vt[:, 0:390], bvt[:], ADD)

    def attn_qk_hi(c, hi):
        # Scores TRANSPOSED [kpos, q], t-major, into a 3-bank tile
        # [t1..t4 | corner_lo | corner_hi]: one 1280-col exp per head.
        tmin = 2 if c == 0 else 1
        tmax = 3 if c == C - 1 else 4
        st = ps_s.tile([128, 1536], f32, tag="S", name="st")
        for t in range(tmin, tmax + 1):
            nc.tensor.matmul(
                st[:, (t - 1) * 256:t * 256],
                k_slice(hi, (2 * (c - 1) + t) * 128, 128),
                q_slice(hi, c * 256, 256),
                start=True, stop=True)
        # edge chunks park their single corner adjacent to the valid
        # interior so softmax is one contiguous exp op
        clo = 768 if c == C - 1 else CLO
        chi = CLO if c == 0 else CHI
        if c > 0:
            nc.tensor.matmul(
                st[:, clo:clo + 128],
                k_slice(hi, (2 * c - 2) * 128, 128),
                q_slice(hi, c * 256, 128),
                start=True, stop=True)
        if c < C - 1:
            nc.tensor.matmul(
                st[:, chi:chi + 128],
                k_slice(hi, (2 * c + 3) * 128, 128),
                q_slice(hi, c * 256 + 128, 128),
                start=True, stop=True)
        ei = e_p.tile([128, 1280], f16, tag="ei", name="ei")
        # exp (scale folds the 1/sqrt(D) q-scaling into the softmax),
        # then triangle masks: t1 queries 128:256 lower-tri, t4 queries
        # 0:128 upper-tri, corners [lower | upper].
        if 0 < c < C - 1:
            nc.scalar.activation(ei[:, 0:PO], st[:, 0:PO],
                                 AEXP, scale=0.125)
            nc.vector.tensor_mul(ei[:, 128:256], ei[:, 128:256],
                                 mask[:, 0:128])
            nc.vector.tensor_mul(ei[:, 768:896], ei[:, 768:896],
                                 mask[:, 128:256])
            nc.vector.tensor_mul(ei[:, CLO:PO], ei[:, CLO:PO],
                                 mask[:, 0:256])
        elif c == 0:
            # valid: t2..t4 [256:1024] + corner [1024:1152]; the two
            # upper-tri masks sit 256 apart -> one strided multiply
            nc.scalar.activation(ei[:, 256:1152], st[:, 256:1152],
                                 AEXP, scale=0.125)
            nc.vector.tensor_tensor(
                ei[:].rearrange("p (a r) -> p a r", r=256)[:, 3:5, 0:128],
                ei[:].rearrange("p (a r) -> p a r", r=256)[:, 3:5, 0:128],
                mask[:, 128:256].rearrange(
                    "p (o r) -> p o r", o=1).broadcast_to([128, 2, 128]),
                MUL)
        else:
            # valid: t1..t3 [0:768] + corner [768:896]; the two
            # lower-tri masks sit 640 apart -> one strided multiply
            nc.scalar.activation(ei[:, 0:896], st[:, 0:896],
                                 AEXP, scale=0.125)
            nc.vector.tensor_tensor(
                ei[:].rearrange("p (a r) -> p a r", r=640)[:, 0:2,
                                                           128:256],
                ei[:].rearrange("p (a r) -> p a r", r=640)[:, 0:2,
                                                           128:256],
                mask[:, 0:128].rearrange(
                    "p (o r) -> p o r", o=1).broadcast_to([128, 2, 128]),
                MUL)
        return ei

    def attn_pv_hi(c, hi, ei, cst):
        tmin = 2 if c == 0 else 1
        tmax = 3 if c == C - 1 else 4
        if hi == 0:
            cst["po"] = ps_p.tile([128, 390], f32, tag="P", name="po")
        po = cst["po"]
        clo = 768 if c == C - 1 else CLO
        chi = CLO if c == 0 else CHI
        for qh in range(2):
            esls = []
            if qh == 0 and c > 0:
                esls.append((ei[:, clo:clo + 128], 2 * (c - 1)))
            for t in range(tmin, tmax + 1):
                esls.append((ei[:, (t - 1) * 256 + qh * 128:
                                (t - 1) * 256 + qh * 128 + 128],
                             2 * (c - 1) + t))
            if qh == 1 and c < C - 1:
                esls.append((ei[:, chi:chi + 128], 2 * c + 3))
            for i, (esl, kt_abs) in enumerate(esls):
                nc.tensor.matmul(
                    po[:, hi * 130 + qh * 65: hi * 130 + qh * 65 + 65],
                    esl,
                    vsb[:, kt_abs * VW + hi * 65:
                        kt_abs * VW + (hi + 1) * 65],
                    start=(i == 0), stop=(i == len(esls) - 1),
                )

    def attn_finish(cst, ots):
        # softmax denominators: one 6-wide reciprocal, then one
        # broadcast multiply rescattering [h][qh][d] -> [qh][h][d].
        # ots rows are padded to 256 cols (512B) so the out DMA runs at
        # full bus rate (<512B elements pay a 2x latency multiplier).
        po = cst["po"]
        rec = e_p.tile([128, 6], f32, tag="rec", name="rec")
        nc.vector.reciprocal(
            rec[:].rearrange("p (g o) -> p g o", o=1),
            po[:].rearrange("p (g d) -> p g d", d=65)[:, :, 64:65])
        nc.vector.tensor_tensor(
            ots[:].rearrange("p (q x) -> p q x", q=2)[:, :, 0:192]
                  .rearrange("p q (h d) -> p q h d", h=HPC),
            po[:].rearrange("p (h q d) -> p q h d", h=HPC, q=2)[:, :, :,
                                                               0:64],
            rec[:].rearrange("p (h q o) -> p q h o", h=HPC, q=2,
                             o=1).broadcast_to([128, 2, HPC, 64]),
            MUL)

    def new_ots():
        return out_p.tile([128, 512], f16, tag="ot", name="ot")

    def dma_out(c, ots, split=False):
        if split:
            nc.sync.dma_start(
                out_d[c * 256:c * 256 + 128, :],
                ots[:, 0:192])
            nc.scalar.dma_start(
                out_d[c * 256 + 128:c * 256 + 256, :],
                ots[:, 192:384])
            return
        nc.sync.dma_start(
            out_d[c * 256:(c + 1) * 256, :].rearrange(
                "(q p) j -> p q j", p=128),
            ots[:].rearrange("p (q j) -> p q j", q=2))

    def emit_round(m, xTn, c1, c2, ce=None, v_extra=None, prefetch=None):
        """One steady-state round for stripe m and chunk pair
        (c1, c2) = (2m-2, 2m-1).  c1 depends only on earlier stripes;
        c2's halo (chunk 2m) lands with this stripe's projection, whose
        pieces are interleaved ahead of it.  v_extra emits a whole
        extra V stripe (round 1 carries V(0)); ce appends a third
        chunk.  V(m)'s second half is deferred until after finish(c1)
        so the 1-bank PSUM ring never cycles into a live buffer.
        prefetch (next stripe's DMA) is issued mid-round so its long
        transfer never queues ahead of this round's small kh2 shift on
        the globally-serial DMA engines."""
        cst1, cst2 = {}, {}
        o1, o2 = new_ots(), new_ots()
        e10 = attn_qk_hi(c1, 0)
        proj_qk_k(m, xTn)
        e11 = attn_qk_hi(c1, 1)
        proj_qk_q(m, xTn)
        e12 = attn_qk_hi(c1, 2)
        if v_extra is not None:
            proj_v(v_extra[0], v_extra[1], 0)
            proj_v(v_extra[0], v_extra[1], 1)
        attn_pv_hi(c1, 0, e10, cst1)
        e20 = attn_qk_hi(c2, 0)
        if prefetch is not None:
            prefetch()
        proj_v(m, xTn, 0)
        attn_pv_hi(c1, 1, e11, cst1)
        e21 = attn_qk_hi(c2, 1)
        attn_pv_hi(c1, 2, e12, cst1)
        attn_finish(cst1, o1)
        if ce is None:
            proj_v(m, xTn, 1)
            e22 = attn_qk_hi(c2, 2)
            dma_out(c1, o1)
            attn_pv_hi(c2, 0, e20, cst2)
            attn_pv_hi(c2, 1, e21, cst2)
            attn_pv_hi(c2, 2, e22, cst2)
            attn_finish(cst2, o2)
            dma_out(c2, o2)
            return
        # ce round: V(m)'s second half is PE filler during the tail
        # chunks' exp backlog; it is only needed by pv(ce).
        e22 = attn_qk_hi(c2, 2)
        dma_out(c1, o1)
        cste = {}
        oe = new_ots()
        attn_pv_hi(c2, 0, e20, cst2)
        ee0 = attn_qk_hi(ce, 0)
        attn_pv_hi(c2, 1, e21, cst2)
        ee1 = attn_qk_hi(ce, 1)
        attn_pv_hi(c2, 2, e22, cst2)
        attn_finish(cst2, o2)
        proj_v(m, xTn, 1)
        ee2 = attn_qk_hi(ce, 2)
        dma_out(c2, o2)
        attn_pv_hi(ce, 0, ee0, cste)
        attn_pv_hi(ce, 1, ee1, cste)
        attn_pv_hi(ce, 2, ee2, cste)
        attn_finish(cste, oe)
        dma_out(ce, oe)

    def emit_round7(m, xTn, c1, c2, ce):
        """Final round: the Act exp chain for chunks c2/ce/15 is the
        kernel's anchor, so chunk c2's scores are pulled as early as
        possible and everything only chunks ce/15 need (Q01(m), both V
        halves) trails as PE filler.  P-ring order: pb, pc, po(c1),
        pa, v0, po(c2), v1, po(ce), po(15)."""
        cst1, cst2, cste = {}, {}, {}
        o1, o2, oe = new_ots(), new_ots(), new_ots()
        e10 = attn_qk_hi(c1, 0)
        proj_qk_k(m, xTn)
        e11 = attn_qk_hi(c1, 1)
        e12 = attn_qk_hi(c1, 2)
        attn_pv_hi(c1, 0, e10, cst1)
        e20 = attn_qk_hi(c2, 0)
        attn_pv_hi(c1, 1, e11, cst1)
        e21 = attn_qk_hi(c2, 1)
        proj_qk_q(m, xTn)
        attn_pv_hi(c1, 2, e12, cst1)
        attn_finish(cst1, o1)
        proj_v(m, xTn, 0)
        e22 = attn_qk_hi(c2, 2)
        dma_out(c1, o1)
        attn_pv_hi(c2, 0, e20, cst2)
        ee0 = attn_qk_hi(ce, 0)
        attn_pv_hi(c2, 1, e21, cst2)
        ee1 = attn_qk_hi(ce, 1)
        attn_pv_hi(c2, 2, e22, cst2)
        attn_finish(cst2, o2)
        proj_v(m, xTn, 1)
        ee2 = attn_qk_hi(ce, 2)
        dma_out(c2, o2)
        attn_pv_hi(ce, 0, ee0, cste)
        attn_pv_hi(ce, 1, ee1, cste)
        attn_pv_hi(ce, 2, ee2, cste)
        attn_finish(cste, oe)
        dma_out(ce, oe)

    def emit_attn_tail(c):
        cst = {}
        es = [attn_qk_hi(c, hi) for hi in range(HPC)]
        o = new_ots()
        for hi in range(HPC):
            attn_pv_hi(c, hi, es[hi], cst)
        # single fused finish + one SP DMA: one HWDGE+DGE latency on
        # the final chain beats two staggered ones
        attn_finish(cst, o)
        dma_out(c, o)

    # ---- pipelined emission ----
    # Startup DMA order (all SP; HWDGE + transfer stages are globally
    # serial, so piece sizing matches the PE's warm-up consumption
    # rate): interleaved wqk / stripe-0 2-k-tile pieces, then consts,
    # then stripes 1-2.  Round m >= 1 runs attention pair (2m-2, 2m-1);
    # round 1 also carries V(0); round 7 carries chunk 14, leaving only
    # chunk 15 for the tail.
    xTn0 = xnat_p.tile([128, KT * 512], f16, tag="xTn", name="xTn0")
    for k0, k1 in ((0, 1), (1, 3), (3, 6)):
        nkt = k1 - k0
        nc.sync.dma_start(
            wqk[:, k0 * 384:k1 * 384].rearrange(
                "p (k c) -> p k c", k=nkt),
            wqk_d[k0 * 128:k1 * 128, :].rearrange(
                "(k p) c -> p k c", p=128))
        nc.sync.dma_start(
            xTn0[:, k0 * 512:k1 * 512].rearrange(
                "p (k c) -> p k c", k=nkt),
            xt_d[k0 * 128:k1 * 128, 0:512].rearrange(
                "(k p) c -> p k c", p=128))
    nc.sync.dma_start(bqk[:], bqk_d[:].rearrange("g p -> p g"))
    nc.sync.dma_start(
        wv[:].rearrange("p (k c) -> p k c", k=KT),
        wv_d[:].rearrange("(k p) c -> p k c", p=128))
    stripes = [xTn0, dma_stripe16(1)]
    nc.sync.dma_start(mask[:], masks_d[:, 0:256])
    nc.sync.dma_start(bvt[:], bvt_d[:])
    proj_qk_stripe0(xTn0)
    proj_v(0, xTn0, 0)
    proj_v(0, xTn0, 1)

    def prefetch(m):
        if m + 1 >= NT:
            return None
        return lambda: stripes.append(dma_stripe16(m + 1))

    for m in range(1, NT - 1):
        emit_round(m, stripes[m], 2 * m - 2, 2 * m - 1,
                   prefetch=prefetch(m))
    emit_round7(NT - 1, stripes[NT - 1], 2 * NT - 4, 2 * NT - 3,
                2 * NT - 2)
    emit_attn_tail(2 * NT - 1)
    ctx.close()


def build_program():
    nc = bacc.Bacc("TRN2", target_bir_lowering=False, debug=False)
    xt_d = nc.dram_tensor("xt", [E, S], f16, kind="ExternalInput").ap()
    wqk_d = nc.dram_tensor("wqk", [E, 384], f16, kind="ExternalInput").ap()
    bqk_d = nc.dram_tensor("bqk", [3, 128], f32, kind="ExternalInput").ap()
    wv_d = nc.dram_tensor("wv", [E, VW], f16, kind="ExternalInput").ap()
    bvt_d = nc.dram_tensor("bvt", [128, 2 * VW], f16,
                           kind="ExternalInput").ap()
    masks_d = nc.dram_tensor("masks", [128, 768], f16,
                             kind="ExternalInput").ap()
    out_d = nc.dram_tensor("out", [S, 256], f16,
                           kind="ExternalOutput").ap()
    with tile.TileContext(nc) as tc:
        _build_body(tc, (xt_d, wqk_d, bqk_d, wv_d, bvt_d, masks_d, out_d))
    nc.compile()
    return nc


def make_in_maps(hidden_states, Wq, bq, Wk, bk, Wv, bv):
    hs = np.asarray(hidden_states, np.float32)
    Wq = np.asarray(Wq, np.float32)
    Wk = np.asarray(Wk, np.float32)
    Wv = np.asarray(Wv, np.float32)
    bq = np.asarray(bq, np.float32)
    bk = np.asarray(bk, np.float32)
    bv = np.asarray(bv, np.float32)

    xts = [np.ascontiguousarray(hs[0].T).astype(np.float16),
           np.ascontiguousarray(hs[1].T).astype(np.float16)]
    tril = np.tril(np.ones((128, 128), np.float16))
    triu = np.triu(np.ones((128, 128), np.float16))
    masks = np.ascontiguousarray(np.concatenate(
        [tril, triu, tril, triu, tril, triu], axis=1))

    in_maps = []
    for core in range(N_CORES):
        b = core // 4
        h0 = HPC * (core % 4)
        wqk = np.concatenate(
            [Wq[:, h0 * 64:(h0 + 2) * 64], Wk[:, h0 * 64:(h0 + 2) * 64],
             Wq[:, (h0 + 2) * 64:(h0 + 3) * 64],
             Wk[:, (h0 + 2) * 64:(h0 + 3) * 64]],
            axis=1).astype(np.float16)
        bqk = np.zeros((3, 128), np.float32)
        bqk[0] = bq[h0 * 64:(h0 + 2) * 64]
        bqk[1] = bk[h0 * 64:(h0 + 2) * 64]
        bqk[2, 0:64] = bq[(h0 + 2) * 64:(h0 + 3) * 64]
        bqk[2, 64:128] = bk[(h0 + 2) * 64:(h0 + 3) * 64]
        wv = np.zeros((E, VW), np.float16)
        bvt1 = np.zeros((VW,), np.float16)
        for i in range(HPC):
            wv[:, 65 * i: 65 * i + 64] = Wv[:, (h0 + i) * 64:
                                            (h0 + i + 1) * 64]
            bvt1[65 * i: 65 * i + 64] = bv[(h0 + i) * 64:(h0 + i + 1) * 64]
            bvt1[65 * i + 64] = 1.0
        bvt = np.broadcast_to(np.concatenate([bvt1, bvt1]),
                              (128, 2 * VW))
        im = {
            "xt": xts[b],
            "bqk": np.ascontiguousarray(bqk),
            "wv": wv,
            "bvt": np.ascontiguousarray(bvt),
            "masks": masks,
            "wqk": np.ascontiguousarray(wqk),
        }
        in_maps.append(im)
    return in_maps


_NC_CACHE = None


def kernel(hidden_states, Wq, bq, Wk, bk, Wv, bv):
    global _NC_CACHE
    if _NC_CACHE is None:
        _NC_CACHE = build_program()
    nc = _NC_CACHE
    in_maps = make_in_maps(hidden_states, Wq, bq, Wk, bk, Wv, bv)
    res = None
    for attempt in range(3):
        try:
            res = bass_utils.run_bass_kernel_spmd(
                nc, in_maps, core_ids=list(range(N_CORES)))
            break
        except Exception:
            if attempt == 2:
                raise
    out = np.zeros((B, S, H * D), np.float32)
    for core in range(N_CORES):
        b = core // 4
        h0 = HPC * (core % 4)
        out[b, :, h0 * 64:(h0 + HPC) * 64] = \
            res.results[core]["out"][:, 0:192]
    return out
